# revision 10
# baseline (speedup 1.0000x reference)
"""Trainium2 Bass kernel for CohereAttention (B=2, S=2048, H=4096, 32Q/8KV heads, D=128).

Sharding: 8 cores = 2 batch groups x 4 tensor-parallel (head) ranks.
Core c: batch b = c // 4, tp rank t = c % 4; owns q-heads [8t, 8t+8),
kv-heads [2t, 2t+2), w_o col slice [1024t, 1024(t+1)).

All matmul operands are bf16 (fp32 PSUM accumulation). Design points:
 - Phase 1 (qkv proj): seq split into 4 groups of 512; full 4096-deep
   contraction accumulates in PSUM (32 chained matmuls, no SBUF adds).
   RoPE (neox-permuted weights, duplicated cos/sin with sign folded into
   sin) applied on PSUM eviction; q/k stay SBUF-resident in transposed
   [d, seq] layout; v tiles are PE-transposed to natural [seq, d] layout
   and also stay resident. No DRAM round-trip for q/k/v.
 - Phase 2 (attention): transposed scores sT[k,q] per 512-wide q group;
   diagonal chunks trim the moving operand to the causal width; exp runs
   on pair-batched [128,1024] PSUM tiles (amortizes ACT overhead);
   softmax denominator = DVE-accumulated sum of p chunks + one
   ones-matmul per (head, group); reciprocal_approx_fast + gpsimd
   partition_broadcast replace the slow DVE reciprocal / PE broadcast.
   Per-(head, group) bf16 AllGather chunks overlap compute and shrink
   the collective tail.
 - Phase 3 (o_proj): w_o preloaded at phase-2 start; per seq-block
   contraction over all 32 gathered head-chunks in PSUM.
"""

from contextlib import ExitStack

import numpy as np

import concourse.bass as bass
import concourse.mybir as mybir
from concourse import bacc
import concourse.tile as tile
from concourse.bass_utils import run_bass_kernel_spmd

B, S, H = 2, 2048, 4096
NQ, NKV, D = 32, 8, 128
THETA = 10000.0
NCORES = 8
TP = 4
QH = NQ // TP               # 8 q heads per core
KH = NKV // TP              # 2 kv heads per core
REP = NQ // NKV             # 4
SCALE = float(D) ** -0.5
QC = QH * D                 # 1024 local q cols
KC = KH * D                 # 256 local k cols
OC = H // TP                # 1024 output cols per core
P = 128
KT = H // P                 # 32 contraction tiles
AKT = NQ * D // P           # 32 contraction tiles for o_proj
NSB = S // P                # 16 seq blocks
SG = 512
NSG = S // SG               # 4 seq groups
NCT = (QC + KC) // P + KH   # 12 col-tiles (8 q + 2 k + 2 v)
NQK = (QC + KC) // P        # 10 q/k col-tiles
F32 = mybir.dt.float32
F32R = mybir.dt.float32r
BF16 = mybir.dt.bfloat16
BF16_NP = mybir.dt.np(mybir.dt.bfloat16)
RG = [[0, 1, 2, 3], [4, 5, 6, 7]]

Exp = mybir.ActivationFunctionType.Exp


def build_program():
    nc = bacc.Bacc('TRN2', target_bir_lowering=False, debug=False, num_devices=NCORES)

    hidT = nc.dram_tensor("hidT", [H, S], BF16, kind="ExternalInput")
    wqkv = nc.dram_tensor("wqkv", [H, NCT * P], BF16, kind="ExternalInput")
    wo = nc.dram_tensor("wo", [NQ * D, OC], BF16, kind="ExternalInput")
    cosf = nc.dram_tensor("cosf", [P, S], F32, kind="ExternalInput")
    sins = nc.dram_tensor("sins", [P, S], F32, kind="ExternalInput")
    ident = nc.dram_tensor("ident", [P, P], BF16, kind="ExternalInput")
    out = nc.dram_tensor("out", [S, OC], F32, kind="ExternalOutput")

    att_d = nc.dram_tensor("att_d", [QH, NSG, P, SG], BF16)
    gath_d = nc.dram_tensor("gath_d", [QH, NSG, TP * P, SG], BF16)

    hid_t = hidT.rearrange("(kt p) s -> p kt s", p=P)     # [128, 32, S]
    wqkv_t = wqkv.rearrange("(kt p) c -> p kt c", p=P)    # [128, 32, 1536]
    wo_t = wo.rearrange("(kt p) c -> p kt c", p=P)        # [128, 32, 1024]

    with tile.TileContext(nc) as tc:
        with tc.tile_pool(name="persist", bufs=1) as prs:
            # q/k transposed [d, seq] and v natural [seq, d], SBUF-resident
            qkT = prs.tile([P, NQK, S], BF16, tag="qkT")
            v_sb = prs.tile([P, NSB, KC], BF16, tag="v")
            cos_sb = prs.tile([P, S], F32, tag="cos")
            sin_sb = prs.tile([P, S], F32, tag="sin")
            id_sb = prs.tile([P, P], BF16, tag="ident")
            nc.sync.dma_start(out=cos_sb[:], in_=cosf[:, :])
            nc.sync.dma_start(out=sin_sb[:], in_=sins[:, :])
            nc.sync.dma_start(out=id_sb[:], in_=ident[:, :])

            # ---------------- Phase 1: qkv projection + RoPE ------------
            with tc.tile_pool(name="ph1_hid", bufs=2) as hidp, \
                 tc.tile_pool(name="ph1_w", bufs=2) as wp, \
                 tc.tile_pool(name="ph1_rope", bufs=2) as rp, \
                 tc.tile_pool(name="ph1_vc", bufs=2) as vcp, \
                 tc.tile_pool(name="ph1_ps", bufs=4, space="PSUM") as pp, \
                 tc.tile_pool(name="ph1_tps", bufs=2, space="PSUM") as tpp:
                for sg in range(NSG):
                    sgs = slice(sg * SG, (sg + 1) * SG)
                    hid_sb = hidp.tile([P, KT, SG], BF16, tag="hid")
                    for kq in range(8):
                        nc.sync.dma_start(
                            out=hid_sb[:, kq * 4:(kq + 1) * 4, :],
                            in_=hid_t[:, kq * 4:(kq + 1) * 4, sgs],
                        )
                    for ct in range(NCT):
                        w_sb = wp.tile([P, KT, P], BF16, tag="w")
                        nc.sync.dma_start(
                            out=w_sb[:], in_=wqkv_t[:, :, ct * P:(ct + 1) * P]
                        )
                        ps = pp.tile([P, SG], F32, tag="ps", name=f"ps_{sg}_{ct}")
                        for kt in range(KT):
                            nc.tensor.matmul(
                                ps[:], w_sb[:, kt, :], hid_sb[:, kt, :],
                                start=(kt == 0), stop=(kt == KT - 1),
                            )
                        if ct < NQK:
                            # RoPE: rot(x) = [-x2; x1], sign folded into sins
                            t1 = rp.tile([P, SG], F32, tag="t1")
                            tmp = rp.tile([P, SG], F32, tag="tmp")
                            nc.scalar.copy(tmp[0:64, :], ps[64:128, :])
                            nc.scalar.copy(tmp[64:128, :], ps[0:64, :])
                            nc.vector.tensor_mul(t1[:], ps[:], cos_sb[:, sgs])
                            nc.vector.tensor_mul(tmp[:], tmp[:], sin_sb[:, sgs])
                            nc.vector.tensor_add(qkT[:, ct, sgs], t1[:], tmp[:])
                        else:
                            # v: cast then PE-transpose to natural layout
                            vc = vcp.tile([P, SG], BF16, tag="vc")
                            nc.scalar.copy(vc[:], ps[:])
                            for nb in range(SG // P):
                                tps = tpp.tile(
                                    [P, P], BF16, tag="tps",
                                    name=f"tps_{sg}_{ct}_{nb}",
                                )
                                nc.tensor.transpose(
                                    tps[:], vc[:, nb * P:(nb + 1) * P], id_sb[:]
                                )
                                nc.scalar.copy(
                                    v_sb[:, sg * 4 + nb,
                                         (ct - NQK) * P:(ct - NQK + 1) * P],
                                    tps[:],
                                )

            # ---------------- Phase 2: attention + Phase 3: o_proj ------
            with tc.tile_pool(name="wo_pool", bufs=1) as wop:
                wo_sb = wop.tile([P, AKT, OC], BF16, tag="wo")
                for kq in range(4):
                    nc.sync.dma_start(
                        out=wo_sb[:, kq * 8:(kq + 1) * 8, :],
                        in_=wo_t[:, kq * 8:(kq + 1) * 8, :],
                    )
                ph2_stack = ExitStack()
                plp = ph2_stack.enter_context(tc.tile_pool(name="ph2_p", bufs=3))
                pcp = ph2_stack.enter_context(tc.tile_pool(name="ph2_pacc", bufs=2))
                smp = ph2_stack.enter_context(tc.tile_pool(name="ph2_sm", bufs=2))
                cp = ph2_stack.enter_context(tc.tile_pool(name="ph2_c", bufs=1))
                sp2 = ph2_stack.enter_context(
                    tc.tile_pool(name="ph2_sp", bufs=2, space="PSUM"))
                ap2 = ph2_stack.enter_context(
                    tc.tile_pool(name="ph2_att", bufs=2, space="PSUM"))
                dp2 = ph2_stack.enter_context(
                    tc.tile_pool(name="ph2_den", bufs=2, space="PSUM"))
                ones_f = cp.tile([P, 1], F32, tag="ones_f")
                nc.vector.memset(ones_f[:], 1.0)
                ones_r = cp.tile([P, 1], F32R, tag="ones_r")
                nc.sync.dma_start(out=ones_r[:], in_=ones_f[:].bitcast(F32R))

                fin = [None]

                def make_fin(qh, j, att_ps, paccs):
                    def _fin():
                        den = dp2.tile([1, SG], F32, tag="den",
                                       name=f"den_{qh}_{j}")
                        nmm = 2 * len(paccs)
                        i = 0
                        for pacc in paccs:
                            for half in (0, 1):
                                nc.tensor.matmul(
                                    den[:], ones_r[:],
                                    pacc[:, half * SG:(half + 1) * SG],
                                    start=(i == 0), stop=(i == nmm - 1),
                                )
                                i += 1
                        rinv = smp.tile([1, SG], F32, tag="rinv")
                        nc.vector.reciprocal_approx_fast(out=rinv[:], in_=den[:])
                        rb = smp.tile([P, SG], F32, tag="rb")
                        nc.gpsimd.partition_broadcast(rb[:], rinv[:])
                        att_sb = smp.tile([P, SG], BF16, tag="attsb")
                        nc.vector.tensor_mul(att_sb[:], att_ps[:], rb[:])
                        nc.sync.dma_start(out=att_d[qh, j], in_=att_sb[:])
                        nc.gpsimd.collective_compute(
                            "AllGather", mybir.AluOpType.bypass,
                            replica_groups=RG,
                            ins=[att_d[qh, j].opt()],
                            outs=[gath_d[qh, j].opt()],
                        )
                    return _fin

                for j in range(NSG):
                    for qh in range(QH):
                        kv = qh // REP
                        ncb = 4 * j + 4
                        npair = ncb // 2
                        # off-diagonal pairs with odd index accumulate the
                        # softmax denominator on gpsimd; the rest on DVE
                        gp_set = {pi for pi in range(2 * j) if pi % 2 == 1}
                        att_ps = ap2.tile([P, SG], F32, tag="att",
                                          name=f"att_{qh}_{j}")
                        pacc_v = pcp.tile([P, 2 * SG], F32R, tag="pv",
                                          name=f"pv_{qh}_{j}")
                        pacc_g = (pcp.tile([P, 2 * SG], F32R, tag="pg",
                                           name=f"pg_{qh}_{j}")
                                  if gp_set else None)
                        pairs = []

                        def chunk_off(c):
                            r = c - 4 * j
                            return 128 * r if r >= 0 else 0

                        def emit_pv(pi):
                            ppair = pairs[pi]
                            for half in (0, 1):
                                c = 2 * pi + half
                                off = chunk_off(c)
                                nc.tensor.matmul(
                                    att_ps[:, off:SG],
                                    v_sb[:, c, kv * P:(kv + 1) * P],
                                    ppair[:, half * SG + off:(half + 1) * SG],
                                    start=(c == 0), stop=(c == ncb - 1),
                                )

                        first_v, first_g = True, True
                        for pi in range(npair):
                            spair = sp2.tile([P, 2 * SG], F32, tag="sp",
                                             name=f"sp_{qh}_{j}_{pi}")
                            ppair = plp.tile([P, 2 * SG], BF16, tag="pp",
                                             name=f"pp_{qh}_{j}_{pi}")
                            pairs.append(ppair)
                            for half in (0, 1):
                                c = 2 * pi + half
                                off = chunk_off(c)
                                nc.tensor.matmul(
                                    spair[:, half * SG + off:(half + 1) * SG],
                                    qkT[:, QH + kv, c * P:(c + 1) * P],
                                    qkT[:, qh, j * SG + off:(j + 1) * SG],
                                    start=True, stop=True,
                                )
                            nc.scalar.activation(ppair[:], spair[:], Exp,
                                                 scale=SCALE)
                            for half in (0, 1):
                                c = 2 * pi + half
                                r = c - 4 * j
                                off = chunk_off(c)
                                if r >= 0:
                                    # zero the causal triangle AND the stale
                                    # region left of the trimmed start, so the
                                    # full-width pair add below is clean
                                    nc.gpsimd.affine_select(
                                        out=ppair[:, half * SG:
                                                  half * SG + off + P],
                                        in_=ppair[:, half * SG:
                                                  half * SG + off + P],
                                        compare_op=mybir.AluOpType.is_ge,
                                        fill=0.0, base=-off,
                                        pattern=[[1, off + P]],
                                        channel_multiplier=-1,
                                    )
                            if pi in gp_set:
                                if first_g:
                                    nc.gpsimd.tensor_copy(pacc_g[:], ppair[:])
                                    first_g = False
                                else:
                                    nc.gpsimd.tensor_add(pacc_g[:], pacc_g[:],
                                                         ppair[:])
                            else:
                                if first_v:
                                    nc.vector.tensor_copy(pacc_v[:], ppair[:])
                                    first_v = False
                                else:
                                    nc.vector.tensor_add(pacc_v[:], pacc_v[:],
                                                         ppair[:])
                            if pi == 0 and fin[0] is not None:
                                fin[0]()
                                fin[0] = None
                            if pi >= 1:
                                emit_pv(pi - 1)
                        emit_pv(npair - 1)
                        paccs = [pacc_v] + ([pacc_g] if pacc_g is not None else [])
                        fin[0] = make_fin(qh, j, att_ps, paccs)
                fin[0]()
                fin[0] = None
                ph2_stack.close()

                # ---------------- Phase 3: o_proj ----------------------
                with tc.tile_pool(name="ph3_g", bufs=2) as gp, \
                     tc.tile_pool(name="ph3_o", bufs=2) as oop, \
                     tc.tile_pool(name="ph3_ps", bufs=2, space="PSUM") as p3:
                    for sb in range(NSB):
                        j, inner = sb // 4, sb % 4
                        g_sb = gp.tile([P, QH, TP, P], BF16, tag="g")
                        for h in range(QH):
                            gd = gath_d[h, j].rearrange("(r p) q -> p r q", p=P)
                            nc.sync.dma_start(
                                out=g_sb[:, h, :, :],
                                in_=gd[:, :, inner * P:(inner + 1) * P],
                            )
                        ps_oc = [
                            p3.tile([P, SG], F32, tag=f"o{oc}",
                                    name=f"o_{sb}_{oc}")
                            for oc in range(2)
                        ]
                        for kt in range(AKT):
                            h, r = kt // 4, kt % 4
                            for oc in range(2):
                                nc.tensor.matmul(
                                    ps_oc[oc][:],
                                    g_sb[:, h, r, :],
                                    wo_sb[:, kt, oc * SG:(oc + 1) * SG],
                                    start=(kt == 0), stop=(kt == AKT - 1),
                                )
                        for oc in range(2):
                            o_sb = oop.tile([P, SG], F32, tag="osb")
                            nc.scalar.copy(o_sb[:], ps_oc[oc][:])
                            nc.sync.dma_start(
                                out=out[sb * P:(sb + 1) * P,
                                        oc * SG:(oc + 1) * SG],
                                in_=o_sb[:],
                            )
    nc.compile()
    return nc


def _prep_inputs(hidden_states, w_qkv, w_o, positions):
    """Host-side sharding + weight permutation. Returns per-core in_maps."""
    hidden_states = np.asarray(hidden_states, dtype=np.float32)
    w_qkv = np.asarray(w_qkv, dtype=np.float32)
    w_o = np.asarray(w_o, dtype=np.float32)
    positions = np.asarray(positions)

    # neox permutation of q/k head columns (evens then odds within each head)
    perm = np.concatenate([np.arange(0, D, 2), np.arange(1, D, 2)])
    wq_all = w_qkv[:, :NQ * D].reshape(H, NQ, D)[:, :, perm]
    wk_all = w_qkv[:, NQ * D:(NQ + NKV) * D].reshape(H, NKV, D)[:, :, perm]
    wv_all = w_qkv[:, (NQ + NKV) * D:].reshape(H, NKV, D)

    # o_proj row permutation: k-tile (h, r) holds global head 8r + h.
    head_order = np.array([8 * r + h for h in range(QH) for r in range(TP)])
    wo_perm = w_o.reshape(NQ, D, H)[head_order]               # [32, D, H]

    inv_freq = 1.0 / (THETA ** (np.arange(0, D, 2, dtype=np.float32) / D))
    identity = np.eye(P, dtype=np.float32).astype(BF16_NP)
    in_maps = []
    for c in range(NCORES):
        b, t = c // TP, c % TP
        freqs = positions[b].astype(np.float32)[None, :] * inv_freq[:, None]
        cos = np.cos(freqs)
        sin = np.sin(freqs)
        cosf = np.concatenate([cos, cos], axis=0).astype(np.float32)
        sins = np.concatenate([-sin, sin], axis=0).astype(np.float32)

        wq = wq_all[:, 8 * t:8 * t + 8].reshape(H, QC)
        wk = wk_all[:, 2 * t:2 * t + 2].reshape(H, KC)
        wv = wv_all[:, 2 * t:2 * t + 2].reshape(H, KC)
        wqkv_c = np.concatenate([wq, wk, wv], axis=1).astype(BF16_NP)
        in_maps.append({
            "hidT": np.ascontiguousarray(hidden_states[b].T).astype(BF16_NP),
            "wqkv": np.ascontiguousarray(wqkv_c),
            "wo": np.ascontiguousarray(
                wo_perm[:, :, 1024 * t:1024 * (t + 1)].reshape(NQ * D, OC)
            ).astype(BF16_NP),
            "cosf": cosf,
            "sins": sins,
            "ident": identity,
        })
    return in_maps


_NC_CACHE = {}


def kernel(hidden_states, w_qkv, w_o, positions, _trace=False):
    if "nc" not in _NC_CACHE:
        _NC_CACHE["nc"] = build_program()
    nc = _NC_CACHE["nc"]
    in_maps = _prep_inputs(hidden_states, w_qkv, w_o, positions)
    res = run_bass_kernel_spmd(nc, in_maps, list(range(NCORES)), trace=_trace)
    out_full = np.empty((B, S, H), dtype=np.float32)
    for c in range(NCORES):
        b, t = c // TP, c % TP
        out_full[b, :, 1024 * t:1024 * (t + 1)] = res.results[c]["out"]
    if _trace:
        kernel.last_exec_time_ns = res.exec_time_ns
        kernel.last_profile = res
    return out_full
